# revision 45
# baseline (speedup 1.0000x reference)
"""Block-local sparse attention with relative position bias on 8 TRN2 NeuronCores.

Sharding: data-parallel over batch (bs=8 == n_cores). Core i computes batch i
end-to-end; weights replicated.

v3 design (vs v1): host-pretransposed x (no DMA transpose), merged dots+G
matmuls (N=504 into one PSUM bank per head), single fused PSUM drains,
SBUF->SBUF sheared DMA for the relative-position skew (no DRAM round trip),
wave-batched elementwise ops, gpsimd offload for the bias add, bf16 output.

Hardcoded problem shapes (self-contained; no reference.py / spec.json reads):
  x (8, 3900, 512) f32, HEADS=8, DH=64, c=200, OFFSET=512.
"""

import math
import sys

import numpy as np

sys.path.insert(0, "/opt/trn_rl_repo")

HEADS = 8
DH = 64
DIM = 512
C = 200
N = 3900
NP = 4000
NB = 20
SEGS = 5
SEG_T = 800  # tokens per segment (4 blocks)
TC = 100  # token chunk (half block)
GW = 304  # padded per-ih G window width (299 -> 304)
SCALE = DH ** -0.5  # 0.125

# merged matmul layout per (hp, up): [grev0 (304) | kt (200) | grev1 (304)]
KTG_W = 808
MM_N = 504  # merged matmul width: ih0 -> cols [0:504], ih1 -> cols [304:808]

# gsb per-block tile (flat row): two ih-major regions of [4w, 2hk, 504].
# Within a (w, hk) slice of region ih: ih0 = [G 304 | dots 200],
# ih1 = [dots 200 | G 304]. G slots are uniform stride 504 within a region.
GSB_ROW = 2 * 4 * 2 * MM_N  # 8064 elems per partition


def build_nc():
    import concourse.bass as bass
    import concourse.mybir as mybir
    import concourse.tile as tile
    from concourse import bacc

    f32 = mybir.dt.float32
    bf16 = mybir.dt.bfloat16
    Exp = mybir.ActivationFunctionType.Exp
    add = mybir.AluOpType.add
    mult = mybir.AluOpType.mult

    nc = bacc.Bacc("TRN2", target_bir_lowering=False, debug=False)

    xT = nc.declare_dram_parameter("xT", [DIM, NP], bf16, isOutput=False)
    wqkt = nc.declare_dram_parameter("wqkt", [128, 4, 1024], bf16, isOutput=False)
    wvt = nc.declare_dram_parameter("wvt", [128, 4, DIM], bf16, isOutput=False)
    woutt = nc.declare_dram_parameter("woutt", [128, 4, DIM], bf16, isOutput=False)
    grevt = nc.declare_dram_parameter("grevt", [128, 2, GW], bf16, isOutput=False)
    idb = nc.declare_dram_parameter("idb", [128, 128], bf16, isOutput=False)
    boutb = nc.declare_dram_parameter("boutb", [1, DIM], bf16, isOutput=False)
    y = nc.declare_dram_parameter("y", [N, DIM], bf16, isOutput=True)
    DBG = globals().get("DEBUG_TAPS", False)
    if DBG:
        dbg_gsb = nc.declare_dram_parameter("dbg_gsb", [128, GSB_ROW], bf16, isOutput=True)
        dbg_pos = nc.declare_dram_parameter("dbg_pos", [TC, 4, C], bf16, isOutput=True)
        dbg_lsb = nc.declare_dram_parameter("dbg_lsb", [TC, 4, C], bf16, isOutput=True)
        dbg_pnb = nc.declare_dram_parameter("dbg_pnb", [TC, 4, C], bf16, isOutput=True)
        dbg_ptb = nc.declare_dram_parameter("dbg_ptb", [TC, 2, 2, C], bf16, isOutput=True)

    xTr = xT.rearrange("(dc p) t -> p dc t", p=128)

    with nc.allow_low_precision("softmax reciprocal/norm in bf16"), tile.TileContext(
        nc
    ) as tc:
        with (
            tc.tile_pool(name="const", bufs=1) as cpool,
            tc.tile_pool(name="seg", bufs=2) as spool,
            tc.tile_pool(name="wave", bufs=3) as wpool,
            tc.tile_pool(name="blk", bufs=2) as bpool,
            tc.tile_pool(name="psum", bufs=1, space="PSUM") as pspool,
        ):
            # ---- constants ----
            wqk_sb = cpool.tile([128, 4, 1024], bf16, tag="wqk")
            wvt_sb = cpool.tile([128, 4, DIM], bf16, tag="wvt")
            wout_sb = cpool.tile([128, 4, DIM], bf16, tag="wout")
            grev_sb = cpool.tile([128, 2, GW], bf16, tag="grev")
            idb_sb = cpool.tile([128, 128], bf16, tag="idb")
            bout_sb = cpool.tile([1, DIM], bf16, tag="bout")
            ones_sb = cpool.tile([1, 128], bf16, tag="ones")
            ktg0 = cpool.tile([128, 4, 4, KTG_W], bf16, tag="ktg0")
            ktg1 = cpool.tile([128, 4, 4, KTG_W], bf16, tag="ktg1")
            ktgs = [ktg0, ktg1]

            # prefetch first segment's x before the weight loads
            xt0 = spool.tile([128, 4, SEG_T], bf16, tag="xt")
            nc.sync.dma_start(out=xt0[:], in_=xTr[:, :, 0:SEG_T])
            nc.sync.dma_start(out=wqk_sb[:], in_=wqkt[:])
            nc.sync.dma_start(out=grev_sb[:], in_=grevt[:])
            nc.sync.dma_start(out=wvt_sb[:], in_=wvt[:])
            nc.sync.dma_start(out=wout_sb[:], in_=woutt[:])
            nc.sync.dma_start(out=idb_sb[:], in_=idb[:])
            nc.sync.dma_start(out=bout_sb[:], in_=boutb[:])
            nc.vector.memset(ones_sb[:], 1.0)
            # warm the ACT exp table during the initial weight DMAs so the
            # first real exp doesn't eat the 1.3us table load
            nc.scalar.activation(
                out=ones_sb[0:1, 0:1], in_=ones_sb[0:1, 0:1], func=Exp
            )
            nc.vector.memset(ones_sb[:], 1.0)

            # fill static grev slots of both ktg buffers (broadcast over hp, up)
            for ktg in ktgs:
                for ih in range(2):
                    src = bass.AP(
                        grev_sb.tensor,
                        grev_sb.offset + ih * GW,
                        [[2 * GW, 128], [0, 4], [0, 4], [1, GW]],
                    )
                    nc.vector.tensor_copy(
                        ktg[:, :, :, ih * MM_N : ih * MM_N + GW], src
                    )


            def copy_on(eng, out, in_):
                if eng is nc.scalar:
                    nc.scalar.copy(out, in_)
                else:
                    eng.tensor_copy(out, in_)

            xt_pref = {0: xt0}

            def proj_segment(s, ktg):
                """Q/K/V projections for segment s (x prefetched a seg early)."""
                t0 = s * SEG_T
                xt = xt_pref.pop(s)
                if s + 1 < SEGS:
                    nxt = spool.tile([128, 4, SEG_T], bf16, tag="xt", name="xtp")
                    nc.sync.dma_start(
                        out=nxt[:], in_=xTr[:, :, t0 + SEG_T : t0 + 2 * SEG_T]
                    )
                    xt_pref[s + 1] = nxt

                qt = spool.tile([128, 4, SEG_T], bf16, tag="qt")
                for oc in range(8):
                    for half in range(2):
                        ps = pspool.tile([128, 512], f32, tag="YP", bufs=2)
                        for dc in range(4):
                            nc.tensor.matmul(
                                ps[:, 0:400],
                                lhsT=wqk_sb[:, dc, oc * 128 : (oc + 1) * 128],
                                rhs=xt[:, dc, half * 400 : (half + 1) * 400],
                                start=(dc == 0),
                                stop=(dc == 3),
                            )
                        eng = nc.vector if (oc + half) % 2 else nc.scalar
                        if oc < 4:
                            copy_on(
                                eng,
                                qt[:, oc, half * 400 : (half + 1) * 400],
                                ps[:, 0:400],
                            )
                        else:
                            hp = oc - 4
                            copy_on(
                                eng,
                                ktg[:, hp, half * 2 : half * 2 + 2, 304:504],
                                ps[:, 0:400].rearrange("p (u t) -> p u t", u=2),
                            )

                vsb = spool.tile([TC, 8, DIM], bf16, tag="vsb")
                for tcn in range(8):
                    ps = pspool.tile([128, 512], f32, tag="YP", bufs=2)
                    for dc in range(4):
                        nc.tensor.matmul(
                            ps[0:TC, :],
                            lhsT=xt[:, dc, tcn * TC : (tcn + 1) * TC],
                            rhs=wvt_sb[:, dc, :],
                            start=(dc == 0),
                            stop=(dc == 3),
                        )
                    eng = nc.vector if tcn % 2 else nc.scalar
                    copy_on(eng, vsb[:, tcn, :], ps[0:TC, :])
                return qt, vsb

            def block_phase1(ug, qt, ktg):
                """block phase 1: merged matmuls .. normalized P (pnb)."""
                up = ug % 4
                last = ug == NB - 1
                ihs = (0,) if last else (0, 1)
                jw = TC if last else C  # key width

                den = bpool.tile([TC, 16], bf16, tag="den")
                rec = bpool.tile([TC, 16], bf16, tag="rec")
                pnbs = []
                nslpw = 2 * len(ihs)  # slots per wave
                gsb = wpool.tile([128, GSB_ROW], bf16, tag="gsb", bufs=2)
                pos = wpool.tile([TC, 2, 8, C], bf16, tag="pos", bufs=2)

                step = 0
                for w in range(4):
                    for ih in ihs:
                        psD = pspool.tile([128, 2, 512], f32, tag="D", bufs=2)
                        for hk in range(2):
                            hr = hk * 64
                            nc.tensor.matmul(
                                psD[0:TC, hk, 0:MM_N],
                                lhsT=qt[
                                    hr : hr + 64,
                                    w,
                                    up * C + ih * TC : up * C + ih * TC + TC,
                                ],
                                rhs=ktg[
                                    hr : hr + 64,
                                    w,
                                    up,
                                    ih * GW : ih * GW + MM_N,
                                ],
                                start=True,
                                stop=True,
                            )
                        # fused drain: [100, 2, 504] -> gsb region (ih, w)
                        eng = nc.scalar if step % 8 in (0, 2, 4, 6, 7) else nc.vector
                        dst = bass.AP(
                            gsb.tensor,
                            gsb.offset + ih * 4032 + w * 1008,
                            [[GSB_ROW, TC], [MM_N, 2], [1, MM_N]],
                        )
                        copy_on(eng, dst, psD[0:TC, :, 0:MM_N])
                        step += 1

                # two sheared SBUF->SBUF DMAs per block (one per ih region):
                # pos[i, ih, (w, hk), r] = G[i, slot, 99-i+r]
                for ih in ihs:
                    goff = ih * 200  # G cols within each 504 slice
                    shear = bass.AP(
                        gsb.tensor,
                        gsb.offset + 99 + ih * 4032 + goff,
                        [[GSB_ROW - 1, TC], [MM_N, 8], [1, C]],
                    )
                    nc.sync.dma_start(out=pos[:, ih, :, :], in_=shear)

                psbs = []
                for w in range(4):
                    nsl = nslpw
                    sl0 = w * nsl
                    # logits = dots + pos
                    lsb = wpool.tile([TC, 4, C], f32, tag="lsb", bufs=2)
                    for ih in ihs:
                        doff = 304 - ih * 304  # dots cols within each 504 slice
                        dots = bass.AP(
                            gsb.tensor,
                            gsb.offset + ih * 4032 + w * 1008 + doff,
                            [[GSB_ROW, TC], [MM_N, 2], [1, jw]],
                        )
                        nc.gpsimd.tensor_tensor(
                            out=lsb[:, ih * 2 : ih * 2 + 2, 0:jw],
                            in0=dots,
                            in1=pos[:, ih, w * 2 : w * 2 + 2, 0:jw],
                            op=add,
                        )

                    # batched exp; denominators via DVE reduce (f32 internal)
                    psb = wpool.tile([TC, 4, C], bf16, tag="psb", bufs=4)
                    nc.scalar.activation(
                        out=psb[:, 0:nsl, 0:jw],
                        in_=lsb[:, 0:nsl, 0:jw],
                        func=Exp,
                    )
                    nc.vector.tensor_reduce(
                        out=den[:, sl0 : sl0 + nsl],
                        in_=psb[:, 0:nsl, 0:jw],
                        axis=mybir.AxisListType.X,
                        op=add,
                    )
                    nc.vector.reciprocal(
                        rec[:, sl0 : sl0 + nsl], den[:, sl0 : sl0 + nsl]
                    )
                    psbs.append(psb)

                # second pass: normalization mults emitted after ALL adds so
                # a mult waiting on recip never blocks the next wave's adds
                # at the head of the in-order Pool queue
                for w in range(4):
                    nsl = nslpw
                    sl0 = w * nsl
                    psb = psbs[w]
                    pnb = wpool.tile([TC, 4, C], bf16, tag="pnb", bufs=6)
                    recb = bass.AP(
                        rec.tensor,
                        rec.offset + sl0,
                        [[16, TC], [1, nsl], [0, jw]],
                    )
                    nc.gpsimd.tensor_tensor(
                        out=pnb[:, 0:nsl, 0:jw],
                        in0=psb[:, 0:nsl, 0:jw],
                        in1=recb,
                        op=mult,
                    )
                    pnbs.append(pnb)
                return pnbs

            def block_phase2(ug, pnbs, vsb, ot):
                """block phase 2: transposes .. attention output (ot)."""
                up = ug % 4
                last = ug == NB - 1
                ihs = (0,) if last else (0, 1)
                njh = 1 if last else 2
                for w in range(4):
                    h0 = 2 * w
                    pnb = pnbs[w]
                    # P^T via PE transposes; pspt cols: ih0 at [0:100], ih1 [100:200]
                    pspt = pspool.tile([128, 2, 2, 256], bf16, tag="T", bufs=1)
                    for hk in range(2):
                        for jh in range(njh):
                            for ih in ihs:
                                nc.tensor.transpose(
                                    pspt[
                                        0:TC, hk, jh, ih * TC : (ih + 1) * TC
                                    ],
                                    in_=pnb[
                                        0:TC, ih * 2 + hk, jh * TC : jh * TC + TC
                                    ],
                                    identity=idb_sb[0:TC, 0:TC],
                                )
                    nq = TC * len(ihs)
                    ptb = wpool.tile([TC, 2, 2, C], bf16, tag="ptb", bufs=2)
                    for hk in range(2):
                        nc.vector.tensor_copy(
                            ptb[:, hk, 0:njh, 0:nq], pspt[0:TC, hk, 0:njh, 0:nq]
                        )

                    pso = pspool.tile([128, 256], f32, tag="O", bufs=1)
                    for hk in range(2):
                        h = h0 + hk
                        for jh in range(njh):
                            nc.tensor.matmul(
                                pso[hk * 64 : hk * 64 + 64, 0:nq],
                                lhsT=vsb[:, up * 2 + jh, h * 64 : (h + 1) * 64],
                                rhs=ptb[:, hk, jh, 0:nq],
                                start=(jh == 0),
                                stop=(jh == njh - 1),
                            )
                    nc.scalar.copy(
                        ot[:, w, up * C : up * C + nq], pso[:, 0:nq]
                    )

            def out_proj(up, k, ot, yt):
                psy = pspool.tile([128, 512], f32, tag="YP", bufs=2)
                for hdc in range(4):
                    nc.tensor.matmul(
                        psy[0:TC, :],
                        lhsT=ot[:, hdc, k * TC : (k + 1) * TC],
                        rhs=wout_sb[:, hdc, :],
                        start=(hdc == 0),
                        stop=False,
                    )
                nc.tensor.matmul(
                    psy[0:TC, :],
                    lhsT=ones_sb[0:1, 0:TC],
                    rhs=bout_sb[0:1, :],
                    start=False,
                    stop=True,
                )
                nc.scalar.copy(yt[:, k % 8, :], psy[0:TC, :])

            seg_state = {}
            pending = None  # (ug, pnbs)

            def emit_phase2(ug):
                s = ug // 4
                up = ug % 4
                vsb, ot, yt = (
                    seg_state[s]["vsb"],
                    seg_state[s]["ot"],
                    seg_state[s]["yt"],
                )
                block_phase2(ug, pending[1], vsb, ot)
                for k in (2 * up, 2 * up + 1):
                    if s * SEG_T + k * TC >= N:
                        continue
                    out_proj(up, k, ot, yt)
                if up == 3 or ug == NB - 1:
                    nk = 8 if s < SEGS - 1 else 7
                    t0 = s * SEG_T
                    # ACT queue: yt writes precede it in-order, so it never
                    # parks the SP queue ahead of the next block's shears
                    nc.scalar.dma_start(
                        out=y[t0 : t0 + nk * TC, :].rearrange(
                            "(k p) d -> p k d", p=TC
                        ),
                        in_=yt[:, 0:nk, :],
                    )

            for ug in range(NB):
                s = ug // 4
                if ug % 4 == 0:
                    qt, vsb = proj_segment(s, ktgs[s % 2])
                    seg_state[s] = dict(
                        qt=qt,
                        vsb=vsb,
                        ot=spool.tile([128, 4, SEG_T], bf16, tag="ot", name="ot"),
                        yt=spool.tile([TC, 8, DIM], bf16, tag="yt", name="yt"),
                    )
                pnbs = block_phase1(ug, seg_state[s]["qt"], ktgs[s % 2])
                if pending is not None:
                    emit_phase2(pending[0])
                pending = (ug, pnbs)
            emit_phase2(pending[0])

    nc.compile()
    return nc


def prep_inputs(x, Wq, Wkv, Wout, bout, rel_emb):
    """Host-side weight re-layouts + padding. Returns per-core in_maps."""
    import ml_dtypes

    x = np.asarray(x, dtype=np.float32)
    Wq = np.asarray(Wq, dtype=np.float32)
    Wkv = np.asarray(Wkv, dtype=np.float32)
    Wout = np.asarray(Wout, dtype=np.float32)
    bout = np.asarray(bout, dtype=np.float32)
    rel_emb = np.asarray(rel_emb, dtype=np.float32)

    bs = x.shape[0]
    bf = ml_dtypes.bfloat16
    # pre-transposed, padded x: (bs, 512, 4000)
    xpad = np.zeros((bs, DIM, NP), dtype=bf)
    xpad[:, :, :N] = x.transpose(0, 2, 1).astype(bf)

    Wk = Wkv[:DIM]
    Wv = Wkv[DIM:]

    def swz(w):  # (512, X) -> (128, 4, X): [p, dc, :] = w[dc*128+p, :]
        return np.ascontiguousarray(
            w.reshape(4, 128, -1).transpose(1, 0, 2)
        ).astype(bf)

    wqkt = swz(np.concatenate([Wq.T * SCALE, Wk.T], axis=1))  # (128, 4, 1024)
    wvt = swz(Wv.T)  # (128, 4, 512)
    woutt = swz(Wout.T)  # (128, 4, 512)

    # Grev[s] = rel_emb[711 - s]; scale already folded into Wq (q pre-scaled)
    grev = rel_emb[711 : 711 - 399 : -1]  # (399, 64)
    grevt = np.zeros((128, 2, GW), dtype=bf)
    grevt[:DH, 0, :299] = grev[100:399].T
    grevt[:DH, 1, :304] = grev[0:304].T
    grevt[DH:, :, :] = grevt[:DH, :, :]

    idb = np.eye(128, dtype=np.float32).astype(bf)
    boutb = np.ascontiguousarray(bout[None, :]).astype(bf)

    in_maps = []
    for b in range(bs):
        in_maps.append(
            dict(
                xT=np.ascontiguousarray(xpad[b]),
                wqkt=wqkt,
                wvt=wvt,
                woutt=woutt,
                grevt=grevt,
                idb=idb,
                boutb=boutb,
            )
        )
    return in_maps


def kernel(x, Wq, Wkv, Wout, bout, rel_emb, context_size=200, **_):
    from concourse.bass_utils import run_bass_kernel_spmd

    in_maps = prep_inputs(x, Wq, Wkv, Wout, bout, rel_emb)
    nc = build_nc()
    res = run_bass_kernel_spmd(nc, in_maps, core_ids=list(range(8)))
    out = np.stack([res.results[b]["y"] for b in range(8)], axis=0)
    return out.astype(np.float32)


if __name__ == "__main__":
    nc = build_nc()
    print("built ok")



# revision 46
# speedup vs baseline: 1.0458x; 1.0458x over previous
"""Block-local sparse attention with relative position bias on 8 TRN2 NeuronCores.

Sharding: data-parallel over batch (bs=8 == n_cores). Core i computes batch i
end-to-end; weights replicated.

v3 design (vs v1): host-pretransposed x (no DMA transpose), merged dots+G
matmuls (N=504 into one PSUM bank per head), single fused PSUM drains,
SBUF->SBUF sheared DMA for the relative-position skew (no DRAM round trip),
wave-batched elementwise ops, gpsimd offload for the bias add, bf16 output.

Hardcoded problem shapes (self-contained; no reference.py / spec.json reads):
  x (8, 3900, 512) f32, HEADS=8, DH=64, c=200, OFFSET=512.
"""

import math
import sys

import numpy as np

sys.path.insert(0, "/opt/trn_rl_repo")

HEADS = 8
DH = 64
DIM = 512
C = 200
N = 3900
NP = 4000
NB = 20
SEGS = 5
SEG_T = 800  # tokens per segment (4 blocks)
TC = 100  # token chunk (half block)
GW = 304  # padded per-ih G window width (299 -> 304)
SCALE = DH ** -0.5  # 0.125

# merged matmul layout per (hp, up): [grev0 (304) | kt (200) | grev1 (304)]
KTG_W = 808
MM_N = 504  # merged matmul width: ih0 -> cols [0:504], ih1 -> cols [304:808]

# gsb per-block tile (flat row): two ih-major regions of [4w, 2hk, 504].
# Within a (w, hk) slice of region ih: ih0 = [G 304 | dots 200],
# ih1 = [dots 200 | G 304]. G slots are uniform stride 504 within a region.
GSB_ROW = 2 * 4 * 2 * MM_N  # 8064 elems per partition


def build_nc():
    import concourse.bass as bass
    import concourse.mybir as mybir
    import concourse.tile as tile
    from concourse import bacc

    f32 = mybir.dt.float32
    bf16 = mybir.dt.bfloat16
    Exp = mybir.ActivationFunctionType.Exp
    add = mybir.AluOpType.add
    mult = mybir.AluOpType.mult

    nc = bacc.Bacc("TRN2", target_bir_lowering=False, debug=False)

    xT = nc.declare_dram_parameter("xT", [DIM, NP], bf16, isOutput=False)
    wqkt = nc.declare_dram_parameter("wqkt", [128, 4, 1024], bf16, isOutput=False)
    wvt = nc.declare_dram_parameter("wvt", [128, 4, DIM], bf16, isOutput=False)
    woutt = nc.declare_dram_parameter("woutt", [128, 4, DIM], bf16, isOutput=False)
    grevt = nc.declare_dram_parameter("grevt", [128, 2, GW], bf16, isOutput=False)
    idb = nc.declare_dram_parameter("idb", [128, 128], bf16, isOutput=False)
    boutb = nc.declare_dram_parameter("boutb", [1, DIM], bf16, isOutput=False)
    y = nc.declare_dram_parameter("y", [N, DIM], bf16, isOutput=True)
    DBG = globals().get("DEBUG_TAPS", False)
    if DBG:
        dbg_gsb = nc.declare_dram_parameter("dbg_gsb", [128, GSB_ROW], bf16, isOutput=True)
        dbg_pos = nc.declare_dram_parameter("dbg_pos", [TC, 4, C], bf16, isOutput=True)
        dbg_lsb = nc.declare_dram_parameter("dbg_lsb", [TC, 4, C], bf16, isOutput=True)
        dbg_pnb = nc.declare_dram_parameter("dbg_pnb", [TC, 4, C], bf16, isOutput=True)
        dbg_ptb = nc.declare_dram_parameter("dbg_ptb", [TC, 2, 2, C], bf16, isOutput=True)

    xTr = xT.rearrange("(dc p) t -> p dc t", p=128)

    with nc.allow_low_precision("softmax reciprocal/norm in bf16"), tile.TileContext(
        nc
    ) as tc:
        with (
            tc.tile_pool(name="const", bufs=1) as cpool,
            tc.tile_pool(name="seg", bufs=2) as spool,
            tc.tile_pool(name="wave", bufs=3) as wpool,
            tc.tile_pool(name="blk", bufs=2) as bpool,
            tc.tile_pool(name="psum", bufs=1, space="PSUM") as pspool,
        ):
            # ---- constants ----
            wqk_sb = cpool.tile([128, 4, 1024], bf16, tag="wqk")
            wvt_sb = cpool.tile([128, 4, DIM], bf16, tag="wvt")
            wout_sb = cpool.tile([128, 4, DIM], bf16, tag="wout")
            grev_sb = cpool.tile([128, 2, GW], bf16, tag="grev")
            idb_sb = cpool.tile([128, 128], bf16, tag="idb")
            bout_sb = cpool.tile([1, DIM], bf16, tag="bout")
            ones_sb = cpool.tile([1, 128], bf16, tag="ones")
            ktg0 = cpool.tile([128, 4, 4, KTG_W], bf16, tag="ktg0")
            ktg1 = cpool.tile([128, 4, 4, KTG_W], bf16, tag="ktg1")
            ktgs = [ktg0, ktg1]

            # prefetch first segment's x before the weight loads
            xt0 = spool.tile([128, 4, SEG_T], bf16, tag="xt")
            nc.sync.dma_start(out=xt0[:], in_=xTr[:, :, 0:SEG_T])
            nc.sync.dma_start(out=wqk_sb[:], in_=wqkt[:])
            nc.sync.dma_start(out=grev_sb[:], in_=grevt[:])
            nc.sync.dma_start(out=wvt_sb[:], in_=wvt[:])
            nc.sync.dma_start(out=wout_sb[:], in_=woutt[:])
            nc.sync.dma_start(out=idb_sb[:], in_=idb[:])
            nc.sync.dma_start(out=bout_sb[:], in_=boutb[:])
            nc.vector.memset(ones_sb[:], 1.0)
            # warm the ACT exp table during the initial weight DMAs so the
            # first real exp doesn't eat the 1.3us table load
            nc.scalar.activation(
                out=ones_sb[0:1, 0:1], in_=ones_sb[0:1, 0:1], func=Exp
            )
            nc.vector.memset(ones_sb[:], 1.0)

            # fill static grev slots of both ktg buffers (broadcast over hp, up)
            for ktg in ktgs:
                for ih in range(2):
                    src = bass.AP(
                        grev_sb.tensor,
                        grev_sb.offset + ih * GW,
                        [[2 * GW, 128], [0, 4], [0, 4], [1, GW]],
                    )
                    nc.vector.tensor_copy(
                        ktg[:, :, :, ih * MM_N : ih * MM_N + GW], src
                    )


            def copy_on(eng, out, in_):
                if eng is nc.scalar:
                    nc.scalar.copy(out, in_)
                else:
                    eng.tensor_copy(out, in_)

            xt_pref = {0: xt0}

            def proj_segment(s, ktg):
                """Q/K/V projections for segment s (x prefetched a seg early)."""
                t0 = s * SEG_T
                xt = xt_pref.pop(s)
                if s + 1 < SEGS:
                    nxt = spool.tile([128, 4, SEG_T], bf16, tag="xt", name="xtp")
                    nc.sync.dma_start(
                        out=nxt[:], in_=xTr[:, :, t0 + SEG_T : t0 + 2 * SEG_T]
                    )
                    xt_pref[s + 1] = nxt

                qt = spool.tile([128, 4, SEG_T], bf16, tag="qt")
                for oc in range(8):
                    for half in range(2):
                        ps = pspool.tile([128, 512], f32, tag="YP", bufs=2)
                        for dc in range(4):
                            nc.tensor.matmul(
                                ps[:, 0:400],
                                lhsT=wqk_sb[:, dc, oc * 128 : (oc + 1) * 128],
                                rhs=xt[:, dc, half * 400 : (half + 1) * 400],
                                start=(dc == 0),
                                stop=(dc == 3),
                            )
                        eng = nc.vector if (oc + half) % 2 else nc.scalar
                        if oc < 4:
                            copy_on(
                                eng,
                                qt[:, oc, half * 400 : (half + 1) * 400],
                                ps[:, 0:400],
                            )
                        else:
                            hp = oc - 4
                            copy_on(
                                eng,
                                ktg[:, hp, half * 2 : half * 2 + 2, 304:504],
                                ps[:, 0:400].rearrange("p (u t) -> p u t", u=2),
                            )

                vsb = spool.tile([TC, 8, DIM], bf16, tag="vsb")
                for tcn in range(8):
                    ps = pspool.tile([128, 512], f32, tag="YP", bufs=2)
                    for dc in range(4):
                        nc.tensor.matmul(
                            ps[0:TC, :],
                            lhsT=xt[:, dc, tcn * TC : (tcn + 1) * TC],
                            rhs=wvt_sb[:, dc, :],
                            start=(dc == 0),
                            stop=(dc == 3),
                        )
                    eng = nc.vector if tcn % 2 else nc.scalar
                    copy_on(eng, vsb[:, tcn, :], ps[0:TC, :])
                return qt, vsb

            def block_phase1(ug, qt, ktg):
                """block phase 1: merged matmuls .. normalized P (pnb)."""
                up = ug % 4
                last = ug == NB - 1
                ihs = (0,) if last else (0, 1)
                jw = TC if last else C  # key width

                den = bpool.tile([TC, 16], bf16, tag="den")
                rec = bpool.tile([TC, 16], bf16, tag="rec")
                pnbs = []
                nslpw = 2 * len(ihs)  # slots per wave
                gsb = wpool.tile([128, GSB_ROW], bf16, tag="gsb", bufs=2)
                pos = wpool.tile([TC, 2, 8, C], bf16, tag="pos", bufs=2)

                step = 0
                for w in range(4):
                    for ih in ihs:
                        psD = pspool.tile([128, 2, 512], f32, tag="D", bufs=2)
                        for hk in range(2):
                            hr = hk * 64
                            nc.tensor.matmul(
                                psD[0:TC, hk, 0:MM_N],
                                lhsT=qt[
                                    hr : hr + 64,
                                    w,
                                    up * C + ih * TC : up * C + ih * TC + TC,
                                ],
                                rhs=ktg[
                                    hr : hr + 64,
                                    w,
                                    up,
                                    ih * GW : ih * GW + MM_N,
                                ],
                                start=True,
                                stop=True,
                            )
                        # fused drain: [100, 2, 504] -> gsb region (ih, w)
                        eng = nc.scalar if step % 8 in (0, 2, 4, 6, 7) else nc.vector
                        dst = bass.AP(
                            gsb.tensor,
                            gsb.offset + ih * 4032 + w * 1008,
                            [[GSB_ROW, TC], [MM_N, 2], [1, MM_N]],
                        )
                        copy_on(eng, dst, psD[0:TC, :, 0:MM_N])
                        step += 1

                # two sheared SBUF->SBUF DMAs per block (one per ih region):
                # pos[i, ih, (w, hk), r] = G[i, slot, 99-i+r]
                for ih in ihs:
                    goff = ih * 200  # G cols within each 504 slice
                    shear = bass.AP(
                        gsb.tensor,
                        gsb.offset + 99 + ih * 4032 + goff,
                        [[GSB_ROW - 1, TC], [MM_N, 8], [1, C]],
                    )
                    nc.sync.dma_start(out=pos[:, ih, :, :], in_=shear)

                psbs = []
                for w in range(4):
                    nsl = nslpw
                    sl0 = w * nsl
                    # logits = dots + pos
                    lsb = wpool.tile([TC, 4, C], f32, tag="lsb", bufs=2)
                    for ih in ihs:
                        doff = 304 - ih * 304  # dots cols within each 504 slice
                        dots = bass.AP(
                            gsb.tensor,
                            gsb.offset + ih * 4032 + w * 1008 + doff,
                            [[GSB_ROW, TC], [MM_N, 2], [1, jw]],
                        )
                        nc.gpsimd.tensor_tensor(
                            out=lsb[:, ih * 2 : ih * 2 + 2, 0:jw],
                            in0=dots,
                            in1=pos[:, ih, w * 2 : w * 2 + 2, 0:jw],
                            op=add,
                        )

                    # batched exp; denominators via DVE reduce (f32 internal)
                    psb = wpool.tile([TC, 4, C], bf16, tag="psb", bufs=2)
                    nc.scalar.activation(
                        out=psb[:, 0:nsl, 0:jw],
                        in_=lsb[:, 0:nsl, 0:jw],
                        func=Exp,
                    )
                    nc.vector.tensor_reduce(
                        out=den[:, sl0 : sl0 + nsl],
                        in_=psb[:, 0:nsl, 0:jw],
                        axis=mybir.AxisListType.X,
                        op=add,
                    )
                    nc.vector.reciprocal(
                        rec[:, sl0 : sl0 + nsl], den[:, sl0 : sl0 + nsl]
                    )
                    psbs.append(psb)

                # second pass: normalization mults emitted after ALL adds so
                # a mult waiting on recip never blocks the next wave's adds
                # at the head of the in-order Pool queue
                for w in range(4):
                    nsl = nslpw
                    sl0 = w * nsl
                    psb = psbs[w]
                    pnb = wpool.tile([TC, 4, C], bf16, tag="pnb", bufs=6)
                    recb = bass.AP(
                        rec.tensor,
                        rec.offset + sl0,
                        [[16, TC], [1, nsl], [0, jw]],
                    )
                    nc.gpsimd.tensor_tensor(
                        out=pnb[:, 0:nsl, 0:jw],
                        in0=psb[:, 0:nsl, 0:jw],
                        in1=recb,
                        op=mult,
                    )
                    pnbs.append(pnb)
                return pnbs

            def block_phase2(ug, pnbs, vsb, ot):
                """block phase 2: transposes .. attention output (ot)."""
                up = ug % 4
                last = ug == NB - 1
                ihs = (0,) if last else (0, 1)
                njh = 1 if last else 2
                for w in range(4):
                    h0 = 2 * w
                    pnb = pnbs[w]
                    # P^T via PE transposes; pspt cols: ih0 at [0:100], ih1 [100:200]
                    pspt = pspool.tile([128, 2, 2, 256], bf16, tag="T", bufs=1)
                    for hk in range(2):
                        for jh in range(njh):
                            for ih in ihs:
                                nc.tensor.transpose(
                                    pspt[
                                        0:TC, hk, jh, ih * TC : (ih + 1) * TC
                                    ],
                                    in_=pnb[
                                        0:TC, ih * 2 + hk, jh * TC : jh * TC + TC
                                    ],
                                    identity=idb_sb[0:TC, 0:TC],
                                )
                    nq = TC * len(ihs)
                    ptb = wpool.tile([TC, 2, 2, C], bf16, tag="ptb", bufs=2)
                    for hk in range(2):
                        nc.vector.tensor_copy(
                            ptb[:, hk, 0:njh, 0:nq], pspt[0:TC, hk, 0:njh, 0:nq]
                        )

                    pso = pspool.tile([128, 256], f32, tag="O", bufs=1)
                    for hk in range(2):
                        h = h0 + hk
                        for jh in range(njh):
                            nc.tensor.matmul(
                                pso[hk * 64 : hk * 64 + 64, 0:nq],
                                lhsT=vsb[:, up * 2 + jh, h * 64 : (h + 1) * 64],
                                rhs=ptb[:, hk, jh, 0:nq],
                                start=(jh == 0),
                                stop=(jh == njh - 1),
                            )
                    nc.scalar.copy(
                        ot[:, w, up * C : up * C + nq], pso[:, 0:nq]
                    )

            def out_proj(up, k, ot, yt):
                psy = pspool.tile([128, 512], f32, tag="YP", bufs=2)
                for hdc in range(4):
                    nc.tensor.matmul(
                        psy[0:TC, :],
                        lhsT=ot[:, hdc, k * TC : (k + 1) * TC],
                        rhs=wout_sb[:, hdc, :],
                        start=(hdc == 0),
                        stop=False,
                    )
                nc.tensor.matmul(
                    psy[0:TC, :],
                    lhsT=ones_sb[0:1, 0:TC],
                    rhs=bout_sb[0:1, :],
                    start=False,
                    stop=True,
                )
                nc.scalar.copy(yt[:, k % 8, :], psy[0:TC, :])

            seg_state = {}
            pending = None  # (ug, pnbs)

            def emit_phase2(ug):
                s = ug // 4
                up = ug % 4
                vsb, ot, yt = (
                    seg_state[s]["vsb"],
                    seg_state[s]["ot"],
                    seg_state[s]["yt"],
                )
                block_phase2(ug, pending[1], vsb, ot)
                for k in (2 * up, 2 * up + 1):
                    if s * SEG_T + k * TC >= N:
                        continue
                    out_proj(up, k, ot, yt)
                if up == 3 or ug == NB - 1:
                    nk = 8 if s < SEGS - 1 else 7
                    t0 = s * SEG_T
                    # ACT queue: yt writes precede it in-order, so it never
                    # parks the SP queue ahead of the next block's shears
                    nc.scalar.dma_start(
                        out=y[t0 : t0 + nk * TC, :].rearrange(
                            "(k p) d -> p k d", p=TC
                        ),
                        in_=yt[:, 0:nk, :],
                    )

            for ug in range(NB):
                s = ug // 4
                if ug % 4 == 0:
                    qt, vsb = proj_segment(s, ktgs[s % 2])
                    seg_state[s] = dict(
                        qt=qt,
                        vsb=vsb,
                        ot=spool.tile([128, 4, SEG_T], bf16, tag="ot", name="ot"),
                        yt=spool.tile([TC, 8, DIM], bf16, tag="yt", name="yt"),
                    )
                pnbs = block_phase1(ug, seg_state[s]["qt"], ktgs[s % 2])
                if pending is not None:
                    emit_phase2(pending[0])
                pending = (ug, pnbs)
            emit_phase2(pending[0])

    nc.compile()
    return nc


def prep_inputs(x, Wq, Wkv, Wout, bout, rel_emb):
    """Host-side weight re-layouts + padding. Returns per-core in_maps."""
    import ml_dtypes

    x = np.asarray(x, dtype=np.float32)
    Wq = np.asarray(Wq, dtype=np.float32)
    Wkv = np.asarray(Wkv, dtype=np.float32)
    Wout = np.asarray(Wout, dtype=np.float32)
    bout = np.asarray(bout, dtype=np.float32)
    rel_emb = np.asarray(rel_emb, dtype=np.float32)

    bs = x.shape[0]
    bf = ml_dtypes.bfloat16
    # pre-transposed, padded x: (bs, 512, 4000)
    xpad = np.zeros((bs, DIM, NP), dtype=bf)
    xpad[:, :, :N] = x.transpose(0, 2, 1).astype(bf)

    Wk = Wkv[:DIM]
    Wv = Wkv[DIM:]

    def swz(w):  # (512, X) -> (128, 4, X): [p, dc, :] = w[dc*128+p, :]
        return np.ascontiguousarray(
            w.reshape(4, 128, -1).transpose(1, 0, 2)
        ).astype(bf)

    wqkt = swz(np.concatenate([Wq.T * SCALE, Wk.T], axis=1))  # (128, 4, 1024)
    wvt = swz(Wv.T)  # (128, 4, 512)
    woutt = swz(Wout.T)  # (128, 4, 512)

    # Grev[s] = rel_emb[711 - s]; scale already folded into Wq (q pre-scaled)
    grev = rel_emb[711 : 711 - 399 : -1]  # (399, 64)
    grevt = np.zeros((128, 2, GW), dtype=bf)
    grevt[:DH, 0, :299] = grev[100:399].T
    grevt[:DH, 1, :304] = grev[0:304].T
    grevt[DH:, :, :] = grevt[:DH, :, :]

    idb = np.eye(128, dtype=np.float32).astype(bf)
    boutb = np.ascontiguousarray(bout[None, :]).astype(bf)

    in_maps = []
    for b in range(bs):
        in_maps.append(
            dict(
                xT=np.ascontiguousarray(xpad[b]),
                wqkt=wqkt,
                wvt=wvt,
                woutt=woutt,
                grevt=grevt,
                idb=idb,
                boutb=boutb,
            )
        )
    return in_maps


def kernel(x, Wq, Wkv, Wout, bout, rel_emb, context_size=200, **_):
    from concourse.bass_utils import run_bass_kernel_spmd

    in_maps = prep_inputs(x, Wq, Wkv, Wout, bout, rel_emb)
    nc = build_nc()
    res = run_bass_kernel_spmd(nc, in_maps, core_ids=list(range(8)))
    out = np.stack([res.results[b]["y"] for b in range(8)], axis=0)
    return out.astype(np.float32)


if __name__ == "__main__":
    nc = build_nc()
    print("built ok")

